# revision 12
# baseline (speedup 1.0000x reference)
"""TRN2 Bass kernel: 100 sequential Linear layers (y = x @ W^T + b).

Data-parallel over 8 NeuronCores: batch 16384 -> 8 shards of 2048 rows;
the 100 [512,512] weight matrices + biases are replicated to every core.

Device layout (per core): activations kept d-major in SBUF as 4 tiles of
[128, 2048] fp32r; each layer is 64 fp32r matmuls (4 j-tiles x 4 batch
chunks of N=512 x 4 k-tiles) accumulated in PSUM, with the bias add fused
into the PSUM->SBUF copy on the vector engine. Host pre-transposes x and
Ws so every DMA is contiguous; fp32r (full-rate fp32 matmul mode, same
bit layout as fp32) is used because plain fp32 matmul runs at 1/4 rate.
"""
import os
import numpy as np

import concourse.bacc as bacc
import concourse.mybir as mybir
import concourse.tile as tile
import concourse.bass_utils as bass_utils
from concourse.bass_utils import run_bass_kernel_spmd

f32 = mybir.dt.float32
f32r = mybir.dt.float32r

N_CORES = 8
N_LAYERS = 100
D = 512
BATCH = 16384
B = BATCH // N_CORES   # 2048 per core
ND = 4                 # contraction tiles of 128
NJ = 4                 # output-row tiles of 128
NB = B // 512          # batch chunks of 512 (one PSUM bank each)

LAST_EXEC_TIME_NS = None
LAST_RESULTS = None

# The axon trace path uploads profile artifacts to a fish bucket that is
# not reachable from this container; keep the artifacts local instead.
bass_utils.upload_artifacts = lambda d: d

_NC_CACHE = {}


def _build_nc(w_bufs=6, copy_engines=("vector", "scalar")):
    nc = bacc.Bacc("TRN2", target_bir_lowering=False, debug=False,
                   num_devices=N_CORES)
    xT = nc.declare_dram_parameter("xT", [D, B], f32r, isOutput=False)
    WT = nc.declare_dram_parameter("WT", [N_LAYERS, D, D], f32r, isOutput=False)
    bsT = nc.declare_dram_parameter("bsT", [128, N_LAYERS * NJ], f32,
                                    isOutput=False)
    yT = nc.declare_dram_parameter("yT", [D, B], f32r, isOutput=True)

    with tile.TileContext(nc) as tc:
        with tc.tile_pool(name="act", bufs=2) as act_pool, \
             tc.tile_pool(name="wpool", bufs=w_bufs) as w_pool, \
             tc.tile_pool(name="bias", bufs=1) as bias_pool, \
             tc.tile_pool(name="ps", bufs=8, space="PSUM") as psum:
            # Head ordering matters: the first matmul needs W0 (1MB) + the
            # first x chunk, so issue W0 on the sync queue first and spread
            # the x loads over the (otherwise idle) gpsimd queue.
            W0 = []
            for d_t in range(ND):
                w = w_pool.tile([128, D], f32r, name=f"W_0_{d_t}", tag=f"W{d_t}")
                nc.sync.dma_start(out=w, in_=WT[0, d_t * 128:(d_t + 1) * 128, :])
                W0.append(w)

            # x loads ride the idle gpsimd queue in 512-wide chunks so the
            # first matmul group's inputs land as early as possible; bias is
            # only needed by the copies, so it goes after the first chunk.
            A = [act_pool.tile([128, B], f32r, name=f"Ain_{d_t}", tag=f"A{d_t}")
                 for d_t in range(ND)]
            bias_sb = bias_pool.tile([128, N_LAYERS * NJ], f32, name="bias_sb")
            for b_c in range(NB):
                for d_t in range(ND):
                    nc.gpsimd.dma_start(
                        out=A[d_t][:, b_c * 512:(b_c + 1) * 512],
                        in_=xT[d_t * 128:(d_t + 1) * 128,
                               b_c * 512:(b_c + 1) * 512])
                if b_c == 0:
                    nc.gpsimd.dma_start(out=bias_sb, in_=bsT[:, :])

            for l in range(N_LAYERS):
                if l == 0:
                    Wl = W0
                else:
                    Wl = []
                    for d_t in range(ND):
                        w = w_pool.tile([128, D], f32r, name=f"W_{l}_{d_t}",
                                        tag=f"W{d_t}")
                        nc.sync.dma_start(out=w,
                                          in_=WT[l, d_t * 128:(d_t + 1) * 128, :])
                        Wl.append(w)
                Anext = [
                    act_pool.tile([128, B], f32r, name=f"A_{l}_{j}", tag=f"A{j}")
                    for j in range(NJ)
                ]
                # b outer: all 4 j-tiles of a batch chunk finish first, so the
                # next layer's matmuls on that chunk start 4 copies in. One
                # PSUM bank per group keeps 8 groups in flight (PE never
                # waits on a slot release).
                for b_c in range(NB):
                    for j_t in range(NJ):
                        ps = psum.tile([128, 512], f32,
                                       name=f"ps_{l}_{j_t}_{b_c}", tag="ps")
                        for d_t in range(ND):
                            nc.tensor.matmul(
                                ps,
                                Wl[d_t][:, j_t * 128:(j_t + 1) * 128],
                                A[d_t][:, b_c * 512:(b_c + 1) * 512],
                                start=(d_t == 0), stop=(d_t == ND - 1))
                        bias_ap = bias_sb[:, l * NJ + j_t:l * NJ + j_t + 1]
                        dst = Anext[j_t][:, b_c * 512:(b_c + 1) * 512]
                        eng = copy_engines[(b_c * NJ + j_t) % len(copy_engines)]
                        if eng == "vector":
                            nc.vector.tensor_scalar_add(out=dst, in0=ps,
                                                        scalar1=bias_ap)
                        else:
                            nc.scalar.add(out=dst, in_=ps, add=bias_ap)
                        if l == N_LAYERS - 1:
                            # stream the result out as soon as it exists
                            nc.sync.dma_start(
                                out=yT[j_t * 128:(j_t + 1) * 128,
                                       b_c * 512:(b_c + 1) * 512],
                                in_=dst)
                A = Anext

    nc.compile()
    return nc


def _get_nc():
    key = "default"
    if key not in _NC_CACHE:
        _NC_CACHE[key] = _build_nc()
    return _NC_CACHE[key]


def kernel(x: np.ndarray, Ws: np.ndarray, bs: np.ndarray) -> np.ndarray:
    global LAST_EXEC_TIME_NS
    x = np.ascontiguousarray(np.asarray(x, dtype=np.float32))
    Ws = np.ascontiguousarray(np.asarray(Ws, dtype=np.float32))
    bs = np.ascontiguousarray(np.asarray(bs, dtype=np.float32))

    # d-major weights: WT[l, d, j] = Ws[l, j, d]
    WT = np.ascontiguousarray(Ws.transpose(0, 2, 1))
    # bias relayout: bsT[p, l*4 + j] = bs[l, j*128 + p]
    bsT = np.ascontiguousarray(
        bs.reshape(N_LAYERS, NJ, 128).transpose(2, 0, 1).reshape(128, N_LAYERS * NJ))

    in_maps = []
    for i in range(N_CORES):
        shard = np.ascontiguousarray(x[i * B:(i + 1) * B, :].T)  # [512, B]
        in_maps.append({"xT": shard, "WT": WT, "bsT": bsT})

    nc = _get_nc()
    trace = os.environ.get("BASS_KERNEL_TRACE", "0") == "1"
    res = run_bass_kernel_spmd(nc, in_maps, list(range(N_CORES)), trace=trace)
    LAST_EXEC_TIME_NS = res.exec_time_ns
    global LAST_RESULTS
    LAST_RESULTS = res

    y = np.concatenate([res.results[i]["yT"].T for i in range(N_CORES)], axis=0)
    return np.ascontiguousarray(y.astype(np.float32))
